# revision 31
# baseline (speedup 1.0000x reference)
"""Multi-head attention layer (B=2,S=2048,D=1024,H=16) on 8 TRN2 NeuronCores.

Sharding: data parallel over batch (2) x tensor parallel over heads (4 heads
per core).  Each core computes, for its (batch b, head-group hg):
  QT = (X_b @ Wq[:,cols] + bq + emotion)^T         [256, S]     (n on partitions)
  KT = (Xv_b @ Wk[:,cols] + bk)^T                  [256, Skv]   (compacted keys)
  V  = Xv_b @ Wv[:,cols] + bv                      [Skv, 256]   (natural, +ones col)
  scoresT[j,i] = KT_h-slices x QT_h, exp fused with 1/8 scale + key mask bias,
  OT_h = V_h_aug^T @ PT  (row 64 = softmax denominator l via the ones column),
  out_partial = (OT/l)^T @ Wo[rows,:]              [S, D]
Host compacts the key/value positions by the attention mask (the padding mask
zeroes whole key columns, so invalid positions are dropped before projection),
then sums the 4 partial outputs per batch and adds bo.

Matmuls run in bfloat16 with fp32 PSUM accumulation: on TRN2 hardware a
bf16 matmul streams 1 row/cycle vs fp32r's 2, and avoids the fp32
high-power duty-cycle throttle.  Measured end-to-end rel err ~7e-3 vs
the fp32 reference (tolerance 2e-2).

Big tensors are host-packed into [128, n*cols] chunk-major layout so every
DMA moves 4KB-contiguous lines, split across the three DGE queues (sync,
scalar, gpsimd) to keep phase-1 loads off each other's critical path.
"""
import math
import sys

sys.path.insert(0, "/opt/trn_rl_repo")

import numpy as np
import ml_dtypes

import concourse.bass as bass
import concourse.tile as tile
from concourse import bacc, mybir
from concourse.bass_utils import run_bass_kernel_spmd

B, S, D, H = 2, 2048, 1024, 16
DH = D // H          # 64
HPC = 4              # heads per core
NCOL = HPC * DH      # 256 columns of Wq/Wk/Wv per core
NC2 = NCOL // 128    # 2 partition-chunks of the head dim
ND = D // 128        # 8 contraction chunks
NI = S // 512        # 4 query 512-chunks
NS = S // 128        # 16 query 128-chunks
F32 = mybir.dt.float32
BF16 = mybir.dt.bfloat16
AF = mybir.ActivationFunctionType

BF16NP = ml_dtypes.bfloat16

_PROGRAM_CACHE = {}


def _chunks(total, step):
    out = []
    o = 0
    while o < total:
        out.append((o, min(step, total - o)))
        o += step
    return out


def build_program(skv: int):
    """One NeuronCore's program; SPMD across 8 cores with different data."""
    nj = skv // 128
    nc = bacc.Bacc("TRN2", target_bir_lowering=False, debug=False, num_devices=8)

    # chunk-major packed layouts: [:, k*cols:(k+1)*cols] is partition-chunk k
    xt = nc.declare_dram_parameter("xt", [128, ND * S], BF16, isOutput=False)
    xtkv = nc.declare_dram_parameter("xtkv", [128, ND * skv], BF16, isOutput=False)
    wq = nc.declare_dram_parameter("wq", [128, ND * NCOL], BF16, isOutput=False)
    wk = nc.declare_dram_parameter("wk", [128, ND * NCOL], BF16, isOutput=False)
    wv = nc.declare_dram_parameter("wv", [128, ND * NCOL], BF16, isOutput=False)
    wo = nc.declare_dram_parameter("wo", [128, NC2 * D], BF16, isOutput=False)
    # packed per-partition scalars: cols [0:2]=bq+ew, [2:4]=bk, [4:4+nj]=mask
    # bias, [4+nj:4+nj+NCOL]=bv broadcast rows
    nsm = 4 + nj + NCOL
    smalls = nc.declare_dram_parameter("smalls", [128, nsm], F32, isOutput=False)
    out = nc.declare_dram_parameter("out", [S, D], BF16, isOutput=True)

    with tile.TileContext(nc) as tc:
        with tc.tile_pool(name="singles", bufs=1) as singles:
            # --- persistent SBUF tiles -----------------------------------
            twqa = singles.tile([128, ND * NCOL], BF16, tag="wqa", name="twqa")
            twka = singles.tile([128, ND * NCOL], BF16, tag="wka", name="twka")
            twva = singles.tile([128, ND * NCOL], BF16, tag="wva", name="twva")
            twoa = singles.tile([128, NC2 * D], BF16, tag="woa", name="twoa")
            txta = singles.tile([128, ND * S], BF16, tag="xta", name="txta")
            txkva = singles.tile([128, ND * skv], BF16, tag="xkva", name="txkva")
            twq = [twqa[:, d * NCOL:(d + 1) * NCOL] for d in range(ND)]
            twk = [twka[:, d * NCOL:(d + 1) * NCOL] for d in range(ND)]
            twv = [twva[:, d * NCOL:(d + 1) * NCOL] for d in range(ND)]
            two = [twoa[:, c * D:(c + 1) * D] for c in range(NC2)]
            txt = [txta[:, d * S:(d + 1) * S] for d in range(ND)]
            txkv = [txkva[:, d * skv:(d + 1) * skv] for d in range(ND)]
            tqt = [singles.tile([128, S], BF16, tag=f"qt{c}", name=f"qt{c}") for c in range(NC2)]
            tkt = [singles.tile([128, skv], BF16, tag=f"kt{c}", name=f"kt{c}") for c in range(NC2)]
            tv = [
                [singles.tile([128, DH + 1], BF16, tag=f"v{h}_{j}", name=f"v{h}_{j}") for j in range(nj)]
                for h in range(HPC)
            ]
            tot = [singles.tile([128, S], F32, tag=f"ot{c}", name=f"ot{c}") for c in range(NC2)]
            totn = [singles.tile([128, S], BF16, tag=f"otn{c}", name=f"otn{c}") for c in range(NC2)]
            # softmax denominators: rows 0/32/64/96 hold heads 0..3
            tstage = singles.tile([97, S], F32, tag="lstage", name="tstage")
            trecf = singles.tile([97, S], F32, tag="lrecf", name="trecf")
            trec = singles.tile([97, S], BF16, tag="lrec", name="trec")
            tones4 = singles.tile([97, 64], BF16, tag="ones4", name="tones4")
            tonesf = singles.tile([128, 64], F32, tag="onesf", name="tonesf")
            tsm = singles.tile([128, 4 + nj + NCOL], F32, tag="smalls", name="tsm")
            tbiasq = [tsm[:, c:c + 1] for c in range(NC2)]
            tbiask = [tsm[:, 2 + c:3 + c] for c in range(NC2)]
            tmb = [tsm[:, 4 + j:5 + j] for j in range(nj)]
            tbvb = tsm[:, 4 + nj:4 + nj + NCOL]

            # --- input DMAs ----------------------------------------------
            # sync queue: first half of the xt stream (Q proj) then the
            # phase-2 out stores.  scalar queue (HWDGE, engine idle in
            # phase 1): wq halves first (Q proj critical path), then the
            # other xt half, then xtkv (K/V proj), then wo.
            # gpsimd queue: small tiles, then wk, wv.
            half = ND // 2
            nc.scalar.dma_start(
                out=twqa[:, : half * NCOL], in_=wq[:, : half * NCOL]
            )
            nc.scalar.dma_start(
                out=twqa[:, half * NCOL:], in_=wq[:, half * NCOL:]
            )
            nc.gpsimd.dma_start(out=tsm, in_=smalls[:, :])
            # xt round-robined over all three queues in consumption order
            # (d ascending) so the Q-proj d-stream never outruns delivery
            xt_eng = [nc.sync, nc.scalar, nc.gpsimd]
            for d in range(ND):
                xt_eng[d % 3].dma_start(
                    out=txt[d], in_=xt[:, d * S:(d + 1) * S]
                )
            for d in range(ND):
                eng = nc.sync if d < half else nc.scalar
                eng.dma_start(
                    out=txkv[d], in_=xtkv[:, d * skv:(d + 1) * skv]
                )
            nc.scalar.dma_start(out=twoa, in_=wo[:, :])
            nc.gpsimd.dma_start(out=twka, in_=wk[:, :])
            nc.gpsimd.dma_start(out=twva, in_=wv[:, :])
            # memset cannot write non-fp32, so round ones through the DVE.
            nc.vector.memset(tonesf, 1.0)
            # per-c reciprocal reads a 33-row band of tstage; only the two
            # head rows are ever written, so init the rest to a safe value
            nc.vector.memset(tstage, 1.0)
            for h in range(HPC):
                nc.vector.tensor_copy(
                    out=tones4[32 * h:32 * h + 1, :], in_=tonesf[0:1, :]
                )

            # --- phase 1: projections (Q, K, V) --------------------------
            with (
                tc.tile_pool(name="pproj", bufs=4, space="PSUM") as pproj,
                tc.tile_pool(name="pv", bufs=2, space="PSUM") as pv,
            ):
                # ones column of V
                for h in range(HPC):
                    for j in range(nj):
                        nc.vector.tensor_copy(
                            out=tv[h][j][:, DH:DH + 1], in_=tonesf[:, 0:1]
                        )
                # QT[n, i]: d-outer so each xt chunk is consumed as it
                # lands and the stationary weight chunk is reused across
                # the 4 query blocks (NI psum banks live)
                for c in range(NC2):
                    for i0 in range(0, NI, 2):
                        pq = [
                            pproj.tile([128, 512], F32, tag="pp", name="pp")
                            for _ in range(2)
                        ]
                        for d in range(ND):
                            for k in range(2):
                                i = i0 + k
                                nc.tensor.matmul(
                                    pq[k],
                                    twq[d][:, c * 128:(c + 1) * 128],
                                    txt[d][:, i * 512:(i + 1) * 512],
                                    start=(d == 0),
                                    stop=(d == ND - 1),
                                )
                        for k in range(2):
                            i = i0 + k
                            nc.vector.tensor_scalar_add(
                                out=tqt[c][:, i * 512:(i + 1) * 512],
                                in0=pq[k],
                                scalar1=tbiasq[c],
                            )
                # KT[n, j]: same d-outer stationary reuse
                kcs = _chunks(skv, 512)
                for c in range(NC2):
                    pk = [
                        pproj.tile([128, 512], F32, tag="pp", name="pp")
                        for _ in kcs
                    ]
                    for d in range(ND):
                        for t, (jo, jw) in enumerate(kcs):
                            nc.tensor.matmul(
                                pk[t][:, 0:jw],
                                twk[d][:, c * 128:(c + 1) * 128],
                                txkv[d][:, jo:jo + jw],
                                start=(d == 0),
                                stop=(d == ND - 1),
                            )
                    for t, (jo, jw) in enumerate(kcs):
                        nc.vector.tensor_scalar_add(
                            out=tkt[c][:, jo:jo + jw],
                            in0=pk[t][:, 0:jw],
                            scalar1=tbiask[c],
                        )
                # V[j, n] accumulated over d, split per head (+bias bv)
                for j in range(nj):
                    ps = pv.tile([128, NCOL], F32, tag="pv", name="pvt")
                    for d in range(ND):
                        nc.tensor.matmul(
                            ps,
                            txkv[d][:, j * 128:(j + 1) * 128],
                            twv[d],
                            start=(d == 0),
                            stop=(d == ND - 1),
                        )
                    for h in range(HPC):
                        nc.vector.tensor_add(
                            out=tv[h][j][:, 0:DH],
                            in0=ps[:, h * DH:(h + 1) * DH],
                            in1=tbvb[:, h * DH:(h + 1) * DH],
                        )

            # --- phase 2: attention + normalize + output projection ------
            # j-loop software-pipelined: attnV(j-1) is emitted after exp(j),
            # so the in-order PE never head-of-line blocks on the exp; each
            # block's normalize + final-projection matmuls are deferred into
            # later j-loops via the pending queue to fill PE bubbles.
            with (
                tc.tile_pool(name="pts", bufs=5) as pts,
                tc.tile_pool(name="obuf", bufs=4) as obuf,
                tc.tile_pool(name="ps2", bufs=2, space="PSUM") as ps2,
                tc.tile_pool(name="pot", bufs=2, space="PSUM") as pot,
                tc.tile_pool(name="plf", bufs=2, space="PSUM") as plf,
            ):
                pending = []

                def emit_norm(i, c):
                    # normalize c-chunk of block i: broadcast 1/l across the
                    # DH partitions via two ones-matmuls packed into one PSUM
                    # bank (partition halves), then write the bf16
                    # normalized OT for the out projection.
                    isl = slice(i * 512, (i + 1) * 512)
                    hA, hB = 2 * c, 2 * c + 1
                    plp = plf.tile([128, 512], F32, tag="plf", name="plp")
                    nc.tensor.matmul(
                        plp[0:64, :],
                        tones4[32 * hA:32 * hA + 1, :],
                        trec[32 * hA:32 * hA + 1, isl],
                        start=True,
                        stop=True,
                        tile_position=(32 * hA, 0),
                    )
                    nc.tensor.matmul(
                        plp[64:128, :],
                        tones4[32 * hB:32 * hB + 1, :],
                        trec[32 * hB:32 * hB + 1, isl],
                        start=True,
                        stop=True,
                        tile_position=(32 * hB, 64),
                    )
                    nc.vector.tensor_mul(
                        out=totn[c][0:64, isl], in0=tot[c][0:64, isl],
                        in1=plp[0:64, :],
                    )
                    nc.vector.tensor_mul(
                        out=totn[c][64:128, isl], in0=tot[c][64:128, isl],
                        in1=plp[64:128, :],
                    )

                def emit_pf(i, so):
                    sidx = i * 4 + so
                    ssl = slice(sidx * 128, (sidx + 1) * 128)
                    for n in range(2):
                        nsl = slice(n * 512, (n + 1) * 512)
                        pf = plf.tile([128, 512], F32, tag="plf", name="pft")
                        for c in range(NC2):
                            nc.tensor.matmul(
                                pf,
                                totn[c][:, ssl],
                                two[c][:, nsl],
                                start=(c == 0),
                                stop=(c == NC2 - 1),
                            )
                        ob = obuf.tile([128, 512], BF16, tag="ob", name="obt")
                        nc.vector.tensor_copy(out=ob, in_=pf)
                        nc.sync.dma_start(out=out[ssl, nsl], in_=ob)

                for i in range(NI):
                    isl = slice(i * 512, (i + 1) * 512)
                    for c in range(NC2):
                        hA, hB = 2 * c, 2 * c + 1
                        potA = pot.tile([DH + 1, 512], F32, tag="pot", name="pott")
                        potB = pot.tile([DH + 1, 512], F32, tag="pot", name="pott")
                        pts_hist = []

                        def emit_scores_exp(j):
                            pscore = ps2.tile(
                                [128, 1024], F32, tag="ps", name="pscore"
                            )
                            nc.tensor.matmul(
                                pscore[:, 0:512],
                                tkt[c][0:64, j * 128:(j + 1) * 128],
                                tqt[c][0:64, isl],
                                start=True,
                                stop=True,
                                tile_position=(0, 0),
                            )
                            nc.tensor.matmul(
                                pscore[:, 512:1024],
                                tkt[c][64:128, j * 128:(j + 1) * 128],
                                tqt[c][64:128, isl],
                                start=True,
                                stop=True,
                                tile_position=(64, 0),
                            )
                            pt = pts.tile([128, 1024], BF16, tag="pt", name="ptile")
                            nc.scalar.activation(
                                out=pt,
                                in_=pscore,
                                func=AF.Exp,
                                bias=tmb[j],
                                scale=1.0 / math.sqrt(DH),
                            )
                            pts_hist.append(pt)

                        def emit_attn(js, last):
                            # same-bank back-to-back accumulation per head to
                            # avoid the PSUM bank-cycling micro-idle penalty
                            for j in js:
                                nc.tensor.matmul(
                                    potA, tv[hA][j], pts_hist[j][:, 0:512],
                                    start=(j == 0),
                                    stop=(last and j == js[-1]),
                                )
                            for j in js:
                                nc.tensor.matmul(
                                    potB, tv[hB][j], pts_hist[j][:, 512:1024],
                                    start=(j == 0),
                                    stop=(last and j == js[-1]),
                                )

                        npair = nj // 2
                        for p in range(npair):
                            emit_scores_exp(2 * p)
                            emit_scores_exp(2 * p + 1)
                            if p > 0:
                                emit_attn((2 * p - 2, 2 * p - 1), last=False)
                            if pending and (
                                p % 2 == 1 or len(pending) > 7 or i == NI - 1
                            ):
                                pending.pop(0)()
                        for j in range(2 * npair, nj):
                            emit_scores_exp(j)
                        tail_js = tuple(range(max(2 * npair - 2, 0), nj))
                        emit_attn(tail_js, last=True)
                        nc.vector.tensor_copy(out=tot[c][0:64, isl], in_=potA[0:DH, :])
                        nc.vector.tensor_copy(out=tot[c][64:128, isl], in_=potB[0:DH, :])
                        nc.vector.tensor_copy(
                            out=tstage[32 * hA:32 * hA + 1, isl],
                            in_=potA[DH:DH + 1, :],
                        )
                        nc.vector.tensor_copy(
                            out=tstage[32 * hB:32 * hB + 1, isl],
                            in_=potB[DH:DH + 1, :],
                        )
                    # batched softmax-denominator reciprocal for the block;
                    # the normalize matmuls + muls are deferred into later
                    # j-loops so the PE never waits on this DVE chain.
                    nc.vector.reciprocal_approx_fast(
                        out=trecf[:, isl], in_=tstage[:, isl]
                    )
                    nc.vector.tensor_copy(out=trec[:, isl], in_=trecf[:, isl])
                    for c in range(NC2):
                        pending.append(lambda i=i, c=c: emit_norm(i, c))
                    for so in range(4):
                        pending.append(lambda i=i, so=so: emit_pf(i, so))
                while pending:
                    pending.pop(0)()

    nc.compile()
    return nc


def _get_program(skv):
    if skv not in _PROGRAM_CACHE:
        _PROGRAM_CACHE[skv] = build_program(skv)
    return _PROGRAM_CACHE[skv]


def _pack_chunks(a, nchunk):
    """[nchunk*128, C] -> [128, nchunk*C] chunk-major (4KB-line DMAs)."""
    c = a.shape[1]
    return np.ascontiguousarray(
        a.reshape(nchunk, 128, c).transpose(1, 0, 2).reshape(128, nchunk * c)
    )


def _pack_smalls(bqe, bkc, bvc, maskb):
    """[128, 4+nj+NCOL]: cols 0:2 bq+ew chunks, 2:4 bk chunks, 4:4+nj mask
    bias columns, 4+nj: bv broadcast."""
    nj = len(maskb) // 128
    sm = np.zeros((128, 4 + nj + NCOL), dtype=np.float32)
    for c in range(NC2):
        sm[:, c] = bqe[c * 128:(c + 1) * 128]
        sm[:, 2 + c] = bkc[c * 128:(c + 1) * 128]
    for j in range(nj):
        sm[:, 4 + j] = maskb[j * 128:(j + 1) * 128]
    sm[:, 4 + nj:] = bvc[None, :]
    return sm


def _shard_inputs(hidden_states, attention_mask, Wq, bq, Wk, bk, Wv, bv,
                  emotion_w, Wo, bo):
    hs = np.asarray(hidden_states, dtype=np.float32)
    mask = np.asarray(attention_mask)
    Wq = np.asarray(Wq, dtype=np.float32)
    Wk = np.asarray(Wk, dtype=np.float32)
    Wv = np.asarray(Wv, dtype=np.float32)
    Wo = np.asarray(Wo, dtype=np.float32)
    bq = np.asarray(bq, dtype=np.float32)
    bk = np.asarray(bk, dtype=np.float32)
    bv = np.asarray(bv, dtype=np.float32)
    ew = np.asarray(emotion_w, dtype=np.float32)

    idx = [np.nonzero(mask[b])[0] for b in range(B)]
    sv = max(len(ix) for ix in idx)
    skv = max(128, ((sv + 127) // 128) * 128)

    in_maps = []
    for b in range(B):
        xt_b = _pack_chunks(hs[b].T.astype(BF16NP), ND)  # [128, ND*S]
        xtkv_f = np.zeros((D, skv), dtype=BF16NP)
        xtkv_f[:, : len(idx[b])] = hs[b][idx[b]].T.astype(BF16NP)
        xtkv_b = _pack_chunks(xtkv_f, ND)
        maskb_b = np.zeros(skv, dtype=np.float32)
        maskb_b[len(idx[b]):] = -1e30
        for hg in range(H // HPC):
            cols = slice(hg * NCOL, (hg + 1) * NCOL)
            in_maps.append(
                {
                    "xt": xt_b,
                    "xtkv": xtkv_b,
                    "wq": _pack_chunks(Wq[:, cols].astype(BF16NP), ND),
                    "wk": _pack_chunks(Wk[:, cols].astype(BF16NP), ND),
                    "wv": _pack_chunks(Wv[:, cols].astype(BF16NP), ND),
                    "wo": _pack_chunks(Wo[cols, :].astype(BF16NP), NC2),
                    "smalls": _pack_smalls(
                        bq[cols] + ew[hg * HPC:(hg + 1) * HPC].reshape(NCOL),
                        bk[cols], bv[cols], maskb_b,
                    ),
                }
            )
    return in_maps, skv, np.asarray(bo, dtype=np.float32)


def run(inputs, trace=False, trace_kwargs=None):
    in_maps, skv, bo = _shard_inputs(**inputs)
    nc = _get_program(skv)
    res = run_bass_kernel_spmd(
        nc,
        in_maps,
        core_ids=list(range(8)),
        trace=trace,
        **(trace_kwargs or {}),
    )
    out = np.zeros((B, S, D), dtype=np.float32)
    for b in range(B):
        acc = np.zeros((S, D), dtype=np.float64)
        for hg in range(4):
            acc += np.asarray(res.results[b * 4 + hg]["out"], dtype=np.float32)
        out[b] = (acc + bo).astype(np.float32)
    return out, res


def kernel(**inputs):
    out, _ = run(inputs, trace=False)
    return out


# revision 33
# speedup vs baseline: 1.2142x; 1.2142x over previous
"""Multi-head attention layer (B=2,S=2048,D=1024,H=16) on 8 TRN2 NeuronCores.

Sharding: data parallel over batch (2) x tensor parallel over heads (4 heads
per core).  Each core computes, for its (batch b, head-group hg):
  QT = (X_b @ Wq[:,cols] + bq + emotion)^T         [256, S]     (n on partitions)
  KT = (Xv_b @ Wk[:,cols] + bk)^T                  [256, Skv]   (compacted keys)
  V  = Xv_b @ Wv[:,cols] + bv                      [Skv, 256]   (natural, +ones col)
  scoresT[j,i] = KT_h-slices x QT_h, exp fused with 1/8 scale + key mask bias,
  OT_h = V_h_aug^T @ PT  (row 64 = softmax denominator l via the ones column),
  out_partial = (OT/l)^T @ Wo[rows,:]              [S, D]
Host compacts the key/value positions by the attention mask (the padding mask
zeroes whole key columns, so invalid positions are dropped before projection),
then sums the 4 partial outputs per batch and adds bo.

Matmuls run in bfloat16 with fp32 PSUM accumulation: on TRN2 hardware a
bf16 matmul streams 1 row/cycle vs fp32r's 2, and avoids the fp32
high-power duty-cycle throttle.  Measured end-to-end rel err ~7e-3 vs
the fp32 reference (tolerance 2e-2).

Big tensors are host-packed into [128, n*cols] chunk-major layout so every
DMA moves 4KB-contiguous lines, split across the three DGE queues (sync,
scalar, gpsimd) to keep phase-1 loads off each other's critical path.
"""
import math
import sys

sys.path.insert(0, "/opt/trn_rl_repo")

import numpy as np
import ml_dtypes

import concourse.bass as bass
import concourse.tile as tile
from concourse import bacc, mybir
from concourse.bass_utils import run_bass_kernel_spmd

B, S, D, H = 2, 2048, 1024, 16
DH = D // H          # 64
HPC = 4              # heads per core
NCOL = HPC * DH      # 256 columns of Wq/Wk/Wv per core
NC2 = NCOL // 128    # 2 partition-chunks of the head dim
ND = D // 128        # 8 contraction chunks
NI = S // 512        # 4 query 512-chunks
NS = S // 128        # 16 query 128-chunks
F32 = mybir.dt.float32
BF16 = mybir.dt.bfloat16
AF = mybir.ActivationFunctionType

BF16NP = ml_dtypes.bfloat16

_PROGRAM_CACHE = {}


def _chunks(total, step):
    out = []
    o = 0
    while o < total:
        out.append((o, min(step, total - o)))
        o += step
    return out


def build_program(skv: int):
    """One NeuronCore's program; SPMD across 8 cores with different data."""
    nj = skv // 128
    nc = bacc.Bacc("TRN2", target_bir_lowering=False, debug=False, num_devices=8)

    # chunk-major packed layouts: [:, k*cols:(k+1)*cols] is partition-chunk k
    xt = nc.declare_dram_parameter("xt", [128, ND * S], BF16, isOutput=False)
    xtkv = nc.declare_dram_parameter("xtkv", [128, ND * skv], BF16, isOutput=False)
    wq = nc.declare_dram_parameter("wq", [128, ND * NCOL], BF16, isOutput=False)
    wk = nc.declare_dram_parameter("wk", [128, ND * NCOL], BF16, isOutput=False)
    wv = nc.declare_dram_parameter("wv", [128, ND * NCOL], BF16, isOutput=False)
    wo = nc.declare_dram_parameter("wo", [128, NC2 * D], BF16, isOutput=False)
    # packed per-partition scalars: cols [0:2]=bq+ew, [2:4]=bk, [4:4+nj]=mask
    # bias, [4+nj:4+nj+NCOL]=bv broadcast rows
    nsm = 4 + nj + NCOL
    smalls = nc.declare_dram_parameter("smalls", [128, nsm], F32, isOutput=False)
    out = nc.declare_dram_parameter("out", [S, D], BF16, isOutput=True)

    with tile.TileContext(nc) as tc:
        with tc.tile_pool(name="singles", bufs=1) as singles:
            # --- persistent SBUF tiles -----------------------------------
            twqa = singles.tile([128, ND * NCOL], BF16, tag="wqa", name="twqa")
            twka = singles.tile([128, ND * NCOL], BF16, tag="wka", name="twka")
            twva = singles.tile([128, ND * NCOL], BF16, tag="wva", name="twva")
            twoa = singles.tile([128, NC2 * D], BF16, tag="woa", name="twoa")
            txta = singles.tile([128, ND * S], BF16, tag="xta", name="txta")
            txkva = singles.tile([128, ND * skv], BF16, tag="xkva", name="txkva")
            twq = [twqa[:, d * NCOL:(d + 1) * NCOL] for d in range(ND)]
            twk = [twka[:, d * NCOL:(d + 1) * NCOL] for d in range(ND)]
            twv = [twva[:, d * NCOL:(d + 1) * NCOL] for d in range(ND)]
            two = [twoa[:, c * D:(c + 1) * D] for c in range(NC2)]
            txt = [txta[:, d * S:(d + 1) * S] for d in range(ND)]
            txkv = [txkva[:, d * skv:(d + 1) * skv] for d in range(ND)]
            tqt = [singles.tile([128, S], BF16, tag=f"qt{c}", name=f"qt{c}") for c in range(NC2)]
            tkt = [singles.tile([128, skv], BF16, tag=f"kt{c}", name=f"kt{c}") for c in range(NC2)]
            tv = [
                [singles.tile([128, DH + 1], BF16, tag=f"v{h}_{j}", name=f"v{h}_{j}") for j in range(nj)]
                for h in range(HPC)
            ]
            tot = [singles.tile([128, S], F32, tag=f"ot{c}", name=f"ot{c}") for c in range(NC2)]
            totn = [singles.tile([128, S], BF16, tag=f"otn{c}", name=f"otn{c}") for c in range(NC2)]
            # softmax denominators: rows 0/32/64/96 hold heads 0..3
            tstage = singles.tile([97, S], F32, tag="lstage", name="tstage")
            trecf = singles.tile([97, S], F32, tag="lrecf", name="trecf")
            trec = singles.tile([97, S], BF16, tag="lrec", name="trec")
            tones4 = singles.tile([97, 64], BF16, tag="ones4", name="tones4")
            tonesf = singles.tile([128, 64], F32, tag="onesf", name="tonesf")
            tsm = singles.tile([128, 4 + nj + NCOL], F32, tag="smalls", name="tsm")
            tbiasq = [tsm[:, c:c + 1] for c in range(NC2)]
            tbiask = [tsm[:, 2 + c:3 + c] for c in range(NC2)]
            tmb = [tsm[:, 4 + j:5 + j] for j in range(nj)]
            tbvb = tsm[:, 4 + nj:4 + nj + NCOL]

            # --- input DMAs ----------------------------------------------
            # sync queue: first half of the xt stream (Q proj) then the
            # phase-2 out stores.  scalar queue (HWDGE, engine idle in
            # phase 1): wq halves first (Q proj critical path), then the
            # other xt half, then xtkv (K/V proj), then wo.
            # gpsimd queue: small tiles, then wk, wv.
            half = ND // 2
            nc.scalar.dma_start(
                out=twqa[:, : half * NCOL], in_=wq[:, : half * NCOL]
            )
            nc.scalar.dma_start(
                out=twqa[:, half * NCOL:], in_=wq[:, half * NCOL:]
            )
            # column halves: the first Q i-pair reads only cols 0:1024 of
            # each chunk, so land all the h0 halves first
            for h in range(2):
                for d in range(half):
                    nc.sync.dma_start(
                        out=txt[d][:, h * 1024:(h + 1) * 1024],
                        in_=xt[:, d * S + h * 1024:d * S + (h + 1) * 1024],
                    )
                for d in range(half, ND):
                    nc.scalar.dma_start(
                        out=txt[d][:, h * 1024:(h + 1) * 1024],
                        in_=xt[:, d * S + h * 1024:d * S + (h + 1) * 1024],
                    )
            for d in range(ND):
                eng = nc.sync if d < half else nc.scalar
                eng.dma_start(
                    out=txkv[d], in_=xtkv[:, d * skv:(d + 1) * skv]
                )
            nc.scalar.dma_start(out=twoa, in_=wo[:, :])
            nc.gpsimd.dma_start(out=tsm, in_=smalls[:, :])
            nc.gpsimd.dma_start(out=twka, in_=wk[:, :])
            nc.gpsimd.dma_start(out=twva, in_=wv[:, :])
            # memset cannot write non-fp32, so round ones through the DVE.
            nc.vector.memset(tonesf, 1.0)
            # per-c reciprocal reads a 33-row band of tstage; only the two
            # head rows are ever written, so init the rest to a safe value
            nc.vector.memset(tstage, 1.0)
            for h in range(HPC):
                nc.vector.tensor_copy(
                    out=tones4[32 * h:32 * h + 1, :], in_=tonesf[0:1, :]
                )

            # --- phase 1: projections (Q, K, V) --------------------------
            with (
                tc.tile_pool(name="pproj", bufs=4, space="PSUM") as pproj,
                tc.tile_pool(name="pv", bufs=2, space="PSUM") as pv,
            ):
                # ones column of V
                for h in range(HPC):
                    for j in range(nj):
                        nc.vector.tensor_copy(
                            out=tv[h][j][:, DH:DH + 1], in_=tonesf[:, 0:1]
                        )
                # QT[n, i]: d-outer so each xt chunk is consumed as it
                # lands and the stationary weight chunk is reused across
                # the 4 query blocks (NI psum banks live)
                for c in range(NC2):
                    for i0 in range(0, NI, 2):
                        pq = [
                            pproj.tile([128, 512], F32, tag="pp", name="pp")
                            for _ in range(2)
                        ]
                        for d in range(ND):
                            for k in range(2):
                                i = i0 + k
                                nc.tensor.matmul(
                                    pq[k],
                                    twq[d][:, c * 128:(c + 1) * 128],
                                    txt[d][:, i * 512:(i + 1) * 512],
                                    start=(d == 0),
                                    stop=(d == ND - 1),
                                )
                        for k in range(2):
                            i = i0 + k
                            nc.vector.tensor_scalar_add(
                                out=tqt[c][:, i * 512:(i + 1) * 512],
                                in0=pq[k],
                                scalar1=tbiasq[c],
                            )
                # KT[n, j]: same d-outer stationary reuse
                kcs = _chunks(skv, 512)
                for c in range(NC2):
                    pk = [
                        pproj.tile([128, 512], F32, tag="pp", name="pp")
                        for _ in kcs
                    ]
                    for d in range(ND):
                        for t, (jo, jw) in enumerate(kcs):
                            nc.tensor.matmul(
                                pk[t][:, 0:jw],
                                twk[d][:, c * 128:(c + 1) * 128],
                                txkv[d][:, jo:jo + jw],
                                start=(d == 0),
                                stop=(d == ND - 1),
                            )
                    for t, (jo, jw) in enumerate(kcs):
                        nc.vector.tensor_scalar_add(
                            out=tkt[c][:, jo:jo + jw],
                            in0=pk[t][:, 0:jw],
                            scalar1=tbiask[c],
                        )
                # V[j, n] accumulated over d, split per head (+bias bv)
                for j in range(nj):
                    ps = pv.tile([128, NCOL], F32, tag="pv", name="pvt")
                    for d in range(ND):
                        nc.tensor.matmul(
                            ps,
                            txkv[d][:, j * 128:(j + 1) * 128],
                            twv[d],
                            start=(d == 0),
                            stop=(d == ND - 1),
                        )
                    for h in range(HPC):
                        nc.vector.tensor_add(
                            out=tv[h][j][:, 0:DH],
                            in0=ps[:, h * DH:(h + 1) * DH],
                            in1=tbvb[:, h * DH:(h + 1) * DH],
                        )

            # --- phase 2: attention + normalize + output projection ------
            # j-loop software-pipelined: attnV(j-1) is emitted after exp(j),
            # so the in-order PE never head-of-line blocks on the exp; each
            # block's normalize + final-projection matmuls are deferred into
            # later j-loops via the pending queue to fill PE bubbles.
            with (
                tc.tile_pool(name="pts", bufs=5) as pts,
                tc.tile_pool(name="obuf", bufs=4) as obuf,
                tc.tile_pool(name="ps2", bufs=2, space="PSUM") as ps2,
                tc.tile_pool(name="pot", bufs=2, space="PSUM") as pot,
                tc.tile_pool(name="plf", bufs=2, space="PSUM") as plf,
            ):
                pending = []

                def emit_norm(i, c):
                    # normalize c-chunk of block i: broadcast 1/l across the
                    # DH partitions via two ones-matmuls packed into one PSUM
                    # bank (partition halves), then write the bf16
                    # normalized OT for the out projection.
                    isl = slice(i * 512, (i + 1) * 512)
                    hA, hB = 2 * c, 2 * c + 1
                    plp = plf.tile([128, 512], F32, tag="plf", name="plp")
                    nc.tensor.matmul(
                        plp[0:64, :],
                        tones4[32 * hA:32 * hA + 1, :],
                        trec[32 * hA:32 * hA + 1, isl],
                        start=True,
                        stop=True,
                        tile_position=(32 * hA, 0),
                    )
                    nc.tensor.matmul(
                        plp[64:128, :],
                        tones4[32 * hB:32 * hB + 1, :],
                        trec[32 * hB:32 * hB + 1, isl],
                        start=True,
                        stop=True,
                        tile_position=(32 * hB, 64),
                    )
                    nc.vector.tensor_mul(
                        out=totn[c][0:64, isl], in0=tot[c][0:64, isl],
                        in1=plp[0:64, :],
                    )
                    nc.vector.tensor_mul(
                        out=totn[c][64:128, isl], in0=tot[c][64:128, isl],
                        in1=plp[64:128, :],
                    )

                def emit_pf(i, so):
                    sidx = i * 4 + so
                    ssl = slice(sidx * 128, (sidx + 1) * 128)
                    for n in range(2):
                        nsl = slice(n * 512, (n + 1) * 512)
                        pf = plf.tile([128, 512], F32, tag="plf", name="pft")
                        for c in range(NC2):
                            nc.tensor.matmul(
                                pf,
                                totn[c][:, ssl],
                                two[c][:, nsl],
                                start=(c == 0),
                                stop=(c == NC2 - 1),
                            )
                        ob = obuf.tile([128, 512], BF16, tag="ob", name="obt")
                        nc.vector.tensor_copy(out=ob, in_=pf)
                        nc.sync.dma_start(out=out[ssl, nsl], in_=ob)

                for i in range(NI):
                    isl = slice(i * 512, (i + 1) * 512)
                    for c in range(NC2):
                        hA, hB = 2 * c, 2 * c + 1
                        potA = pot.tile([DH + 1, 512], F32, tag="pot", name="pott")
                        potB = pot.tile([DH + 1, 512], F32, tag="pot", name="pott")
                        pts_hist = []

                        def emit_scores_exp(j):
                            pscore = ps2.tile(
                                [128, 1024], F32, tag="ps", name="pscore"
                            )
                            nc.tensor.matmul(
                                pscore[:, 0:512],
                                tkt[c][0:64, j * 128:(j + 1) * 128],
                                tqt[c][0:64, isl],
                                start=True,
                                stop=True,
                                tile_position=(0, 0),
                            )
                            nc.tensor.matmul(
                                pscore[:, 512:1024],
                                tkt[c][64:128, j * 128:(j + 1) * 128],
                                tqt[c][64:128, isl],
                                start=True,
                                stop=True,
                                tile_position=(64, 0),
                            )
                            pt = pts.tile([128, 1024], BF16, tag="pt", name="ptile")
                            nc.scalar.activation(
                                out=pt,
                                in_=pscore,
                                func=AF.Exp,
                                bias=tmb[j],
                                scale=1.0 / math.sqrt(DH),
                            )
                            pts_hist.append(pt)

                        def emit_attn(js, last):
                            # same-bank back-to-back accumulation per head to
                            # avoid the PSUM bank-cycling micro-idle penalty
                            for j in js:
                                nc.tensor.matmul(
                                    potA, tv[hA][j], pts_hist[j][:, 0:512],
                                    start=(j == 0),
                                    stop=(last and j == js[-1]),
                                )
                            for j in js:
                                nc.tensor.matmul(
                                    potB, tv[hB][j], pts_hist[j][:, 512:1024],
                                    start=(j == 0),
                                    stop=(last and j == js[-1]),
                                )

                        npair = nj // 2
                        for p in range(npair):
                            emit_scores_exp(2 * p)
                            emit_scores_exp(2 * p + 1)
                            if p > 0:
                                emit_attn((2 * p - 2, 2 * p - 1), last=False)
                            if pending and (
                                p % 2 == 1 or len(pending) > 7 or i == NI - 1
                            ):
                                pending.pop(0)()
                        for j in range(2 * npair, nj):
                            emit_scores_exp(j)
                        tail_js = tuple(range(max(2 * npair - 2, 0), nj))
                        emit_attn(tail_js, last=True)
                        nc.vector.tensor_copy(out=tot[c][0:64, isl], in_=potA[0:DH, :])
                        nc.vector.tensor_copy(out=tot[c][64:128, isl], in_=potB[0:DH, :])
                        nc.vector.tensor_copy(
                            out=tstage[32 * hA:32 * hA + 1, isl],
                            in_=potA[DH:DH + 1, :],
                        )
                        nc.vector.tensor_copy(
                            out=tstage[32 * hB:32 * hB + 1, isl],
                            in_=potB[DH:DH + 1, :],
                        )
                    # batched softmax-denominator reciprocal for the block;
                    # the normalize matmuls + muls are deferred into later
                    # j-loops so the PE never waits on this DVE chain.
                    nc.vector.reciprocal_approx_fast(
                        out=trecf[:, isl], in_=tstage[:, isl]
                    )
                    nc.vector.tensor_copy(out=trec[:, isl], in_=trecf[:, isl])
                    for c in range(NC2):
                        pending.append(lambda i=i, c=c: emit_norm(i, c))
                    for so in range(4):
                        pending.append(lambda i=i, so=so: emit_pf(i, so))
                while pending:
                    pending.pop(0)()

    nc.compile()
    return nc


def _get_program(skv):
    if skv not in _PROGRAM_CACHE:
        _PROGRAM_CACHE[skv] = build_program(skv)
    return _PROGRAM_CACHE[skv]


def _pack_chunks(a, nchunk):
    """[nchunk*128, C] -> [128, nchunk*C] chunk-major (4KB-line DMAs)."""
    c = a.shape[1]
    return np.ascontiguousarray(
        a.reshape(nchunk, 128, c).transpose(1, 0, 2).reshape(128, nchunk * c)
    )


def _pack_smalls(bqe, bkc, bvc, maskb):
    """[128, 4+nj+NCOL]: cols 0:2 bq+ew chunks, 2:4 bk chunks, 4:4+nj mask
    bias columns, 4+nj: bv broadcast."""
    nj = len(maskb) // 128
    sm = np.zeros((128, 4 + nj + NCOL), dtype=np.float32)
    for c in range(NC2):
        sm[:, c] = bqe[c * 128:(c + 1) * 128]
        sm[:, 2 + c] = bkc[c * 128:(c + 1) * 128]
    for j in range(nj):
        sm[:, 4 + j] = maskb[j * 128:(j + 1) * 128]
    sm[:, 4 + nj:] = bvc[None, :]
    return sm


def _shard_inputs(hidden_states, attention_mask, Wq, bq, Wk, bk, Wv, bv,
                  emotion_w, Wo, bo):
    hs = np.asarray(hidden_states, dtype=np.float32)
    mask = np.asarray(attention_mask)
    Wq = np.asarray(Wq, dtype=np.float32)
    Wk = np.asarray(Wk, dtype=np.float32)
    Wv = np.asarray(Wv, dtype=np.float32)
    Wo = np.asarray(Wo, dtype=np.float32)
    bq = np.asarray(bq, dtype=np.float32)
    bk = np.asarray(bk, dtype=np.float32)
    bv = np.asarray(bv, dtype=np.float32)
    ew = np.asarray(emotion_w, dtype=np.float32)

    idx = [np.nonzero(mask[b])[0] for b in range(B)]
    sv = max(len(ix) for ix in idx)
    skv = max(128, ((sv + 127) // 128) * 128)

    in_maps = []
    for b in range(B):
        xt_b = _pack_chunks(hs[b].T.astype(BF16NP), ND)  # [128, ND*S]
        xtkv_f = np.zeros((D, skv), dtype=BF16NP)
        xtkv_f[:, : len(idx[b])] = hs[b][idx[b]].T.astype(BF16NP)
        xtkv_b = _pack_chunks(xtkv_f, ND)
        maskb_b = np.zeros(skv, dtype=np.float32)
        maskb_b[len(idx[b]):] = -1e30
        for hg in range(H // HPC):
            cols = slice(hg * NCOL, (hg + 1) * NCOL)
            in_maps.append(
                {
                    "xt": xt_b,
                    "xtkv": xtkv_b,
                    "wq": _pack_chunks(Wq[:, cols].astype(BF16NP), ND),
                    "wk": _pack_chunks(Wk[:, cols].astype(BF16NP), ND),
                    "wv": _pack_chunks(Wv[:, cols].astype(BF16NP), ND),
                    "wo": _pack_chunks(Wo[cols, :].astype(BF16NP), NC2),
                    "smalls": _pack_smalls(
                        bq[cols] + ew[hg * HPC:(hg + 1) * HPC].reshape(NCOL),
                        bk[cols], bv[cols], maskb_b,
                    ),
                }
            )
    return in_maps, skv, np.asarray(bo, dtype=np.float32)


def run(inputs, trace=False, trace_kwargs=None):
    in_maps, skv, bo = _shard_inputs(**inputs)
    nc = _get_program(skv)
    res = run_bass_kernel_spmd(
        nc,
        in_maps,
        core_ids=list(range(8)),
        trace=trace,
        **(trace_kwargs or {}),
    )
    out = np.zeros((B, S, D), dtype=np.float32)
    for b in range(B):
        acc = np.zeros((S, D), dtype=np.float64)
        for hg in range(4):
            acc += np.asarray(res.results[b * 4 + hg]["out"], dtype=np.float32)
        out[b] = (acc + bo).astype(np.float32)
    return out, res


def kernel(**inputs):
    out, _ = run(inputs, trace=False)
    return out
